# revision 14
# baseline (speedup 1.0000x reference)
"""Trainium2 Bass kernel for BarlowTwinsLoss (nn_BarlowTwinsLoss_11038065951192).

Full inputs: e_q, tau [16384, 2048] f32. Output: scalar f32 loss.

Strategy (data-parallel over the batch axis, 8 NeuronCores):
  - each core holds a [2048, 2048] row-shard of e_q and tau
  - one pass over the shard computes 5 per-feature partial sums in PSUM via
    ones-vector matmuls: S1e, S1t, S2e, S2t, Set (length-2048 each; matmul
    outputs may only target PSUM partitions {0,32,64}, so stats pack as
    partition row 32*g with two 2048-wide stat slots per row)
  - each core DMAs its raw 5x2048 partial stats to DRAM ([3, 4096] f32);
    the host sums the 8 cores' partials in f64 and evaluates the tiny
    closed-form epilogue (mean/std/diag-corr/loss) — that is the
    gather/unshard step for this batch-sharded loss. No collective, no
    on-device epilogue: the kernel is a single streaming pass.

Pipeline balance per [128, 2048] tile (HW-measured via loop-diff bench):
  DMA ~4.3us (2 MiB folded transfers, FOLD=2), PE 20x N=512 ones-matmuls
  ~4.4us, ACT 2 squares ~3.8us, DVE 2 bf16 casts + 1 mul ~3.4us — all
  overlapped by the Tile scheduler; whole pass ~88us vs a ~70-76us
  pure-DMA floor measured on these cores. Tail tricks: the last tau tile's
  DMA is split so the final dependency chain is short; PSUM->SBUF staging
  is issued per 512-wide bank group (DVE/ACT alternating) so it overlaps
  the last matmuls; the out-DMA rides the ACT HWDGE ring so its staging
  semaphore wait cannot stall the SP ring's input stream (the rings are
  FIFO), which matters when the kernel runs back-to-back.

The module is self-contained: it builds + compiles the Bass graph on first
call and caches the jitted PJRT executable for repeat calls.

Hardware pitfalls baked into this design (found by probing; the simulator
accepts all of them but silicon does not):
  - DVE tensor_tensor with f32 inputs and bf16 output produces garbage ->
    multiply the bf16 copies instead
  - InstTensorTensorReduce crashes the exec unit -> tensor_mul + reduce_sum
  - ACT reading bf16 input crashes the exec unit -> keep ACT on f32 inputs
  - DMA cannot read PSUM -> stage through SBUF with a compute-engine copy
"""

import numpy as np

N_FULL = 16384
D = 2048
N_CORES = 8
N_SHARD = N_FULL // N_CORES  # 2048 rows per core
P = 128
N_TILES = N_SHARD // P  # 16
CHUNK = 512
N_CHUNKS = D // CHUNK  # 4
FOLD = 2  # row-tiles per DMA: 2 MiB transfers sustain ~512 GB/s vs ~320 at 1 MiB
EPS = 1e-9

_CACHE = {}


def _build_nc(repeat=1, loop=None):
    import contextlib

    import concourse.bacc as bacc
    import concourse.tile as tile
    from concourse import mybir

    f32 = mybir.dt.float32
    bf16 = mybir.dt.bfloat16
    Act = mybir.ActivationFunctionType

    nc = bacc.Bacc(
        "TRN2",
        target_bir_lowering=False,
        debug=False,
        enable_asserts=False,
        num_devices=N_CORES,
    )
    eq_d = nc.dram_tensor("e_q", [N_SHARD, D], f32, kind="ExternalInput")
    ta_d = nc.dram_tensor("tau", [N_SHARD, D], f32, kind="ExternalInput")
    out_d = nc.dram_tensor("out", [3, 2 * D], f32, kind="ExternalOutput")

    with tile.TileContext(nc) as tc:
        with (
            tc.tile_pool(name="io", bufs=3 if FOLD <= 2 else 2) as io,
            tc.tile_pool(name="bfp", bufs=2) as bfp,
            tc.tile_pool(name="misc", bufs=1) as misc,
            tc.tile_pool(name="ep", bufs=1) as ep,
            tc.tile_pool(name="psp", bufs=1, space="PSUM") as psp,
        ):
            ones_bf = misc.tile([P, 1], bf16)
            nc.gpsimd.memset(ones_bf[:], 1.0)
            zero_b = misc.tile([P, 1], f32)
            nc.gpsimd.memset(zero_b[:], 0.0)

            # stats accumulate in PSUM; matmuls only write rows {0,32,64} --
            # zero the tile once so the whole-tile PSUM->SBUF staging copy
            # reads initialized memory (start=True re-inits written regions
            # on every pass).
            psum_stats = psp.tile([65, 2 * N_CHUNKS * CHUNK], f32, tag="stats")
            nc.vector.memset(psum_stats[:], 0.0)

            for _rep in range(repeat):
                loop_cm = (
                    tc.For_i(
                        0,
                        loop,
                        1,
                        hint_engines=(
                            mybir.EngineType.PE,
                            mybir.EngineType.DVE,
                            mybir.EngineType.Activation,
                            mybir.EngineType.SP,
                        ),
                    )
                    if loop is not None
                    else contextlib.nullcontext()
                )
                with contextlib.ExitStack() as _stack:
                    _stack.enter_context(loop_cm)

                    e_big = t_big = None
                    for i in range(N_TILES):
                        last = i == N_TILES - 1
                        half = i % FOLD
                        if half == 0:
                            # one folded DMA covers FOLD row-tiles: row
                            # c*128+p of the slab lands at (p, c*D + j).
                            pair = i // FOLD
                            rows = P * FOLD
                            e_big = io.tile([P, FOLD * D], f32, tag="e")
                            t_big = io.tile([P, FOLD * D], f32, tag="t")
                            if FOLD == 1:
                                nc.sync.dma_start(
                                    e_big[:],
                                    eq_d[pair * rows : (pair + 1) * rows, :],
                                )
                            else:
                                src_e = eq_d[
                                    pair * rows : (pair + 1) * rows, :
                                ].rearrange("(c p) j -> p c j", c=FOLD)
                                nc.sync.dma_start(
                                    e_big[:].rearrange(
                                        "p (c j) -> p c j", c=FOLD
                                    ),
                                    src_e,
                                )
                            if pair == N_TILES // FOLD - 1:
                                # final tau transfer gates the whole tail:
                                # issue it per row-tile, with the very last
                                # tile split in half so the dependent chain
                                # after the final piece is short.
                                for c in range(FOLD):
                                    dst = t_big[:, c * D : (c + 1) * D]
                                    src = ta_d[
                                        pair * rows + c * P :
                                        pair * rows + (c + 1) * P, :
                                    ]
                                    if c == FOLD - 1:
                                        h = D // 2
                                        nc.sync.dma_start(dst[:, :h], src[:, :h])
                                        nc.sync.dma_start(dst[:, h:], src[:, h:])
                                    else:
                                        nc.sync.dma_start(dst, src)
                            elif FOLD == 1:
                                nc.sync.dma_start(
                                    t_big[:],
                                    ta_d[pair * rows : (pair + 1) * rows, :],
                                )
                            else:
                                src_t = ta_d[
                                    pair * rows : (pair + 1) * rows, :
                                ].rearrange("(c p) j -> p c j", c=FOLD)
                                nc.sync.dma_start(
                                    t_big[:].rearrange(
                                        "p (c j) -> p c j", c=FOLD
                                    ),
                                    src_t,
                                )
                        e_t = e_big[:, half * D : (half + 1) * D]
                        t_t = t_big[:, half * D : (half + 1) * D]

                        e_bf = bfp.tile([P, D], bf16, tag="e_bf")
                        t_bf = bfp.tile([P, D], bf16, tag="t_bf")
                        e2_bf = bfp.tile([P, D], bf16, tag="e2_bf")
                        t2_bf = bfp.tile([P, D], bf16, tag="t2_bf")
                        et_bf = bfp.tile([P, D], bf16, tag="et_bf")

                        nc.vector.tensor_copy(e_bf[:], e_t[:])
                        nc.scalar.activation(
                            e2_bf[:], e_t[:], Act.Square, bias=zero_b[:]
                        )
                        if last:
                            h = D // 2
                            for a, b in ((0, h), (h, D)):
                                nc.vector.tensor_copy(t_bf[:, a:b], t_t[:, a:b])
                                nc.scalar.activation(
                                    t2_bf[:, a:b],
                                    t_t[:, a:b],
                                    Act.Square,
                                    bias=zero_b[:],
                                )
                                nc.vector.tensor_mul(
                                    et_bf[:, a:b], e_bf[:, a:b], t_bf[:, a:b]
                                )
                        else:
                            nc.vector.tensor_copy(t_bf[:], t_t[:])
                            nc.scalar.activation(
                                t2_bf[:], t_t[:], Act.Square, bias=zero_b[:]
                            )
                            nc.vector.tensor_mul(et_bf[:], e_bf[:], t_bf[:])

                        # stream order e, e2, t, t2, et: everything gated on
                        # the late-arriving tau tile issues as one dense PE
                        # burst at the end, with no earlier-ready work queued
                        # behind it.
                        for s, src in ((0, e_bf), (2, e2_bf), (1, t_bf),
                                       (3, t2_bf), (4, et_bf)):
                            g, sl = divmod(s, 2)
                            for c in range(N_CHUNKS):
                                col = (sl * N_CHUNKS + c) * CHUNK
                                nc.tensor.matmul(
                                    psum_stats[
                                        32 * g : 32 * g + 1, col : col + CHUNK
                                    ],
                                    ones_bf[:, 0:1],
                                    src[:, c * CHUNK : (c + 1) * CHUNK],
                                    start=(i == 0),
                                    stop=(i == N_TILES - 1),
                                )

                    # PSUM -> SBUF staging (DMA cannot read PSUM). Stage per
                    # 512-wide bank group, alternating DVE/ACT, so each copy
                    # starts as soon as the final matmul touching its column
                    # range completes (the Tile tracker gates per region).
                    sb_stats = ep.tile(
                        [65, 2 * N_CHUNKS * CHUNK], f32, tag="sb_stats"
                    )
                    for c in range(N_CHUNKS):
                        lo = c * CHUNK
                        hi = N_CHUNKS * CHUNK + c * CHUNK
                        nc.scalar.copy(
                            sb_stats[:, lo : lo + CHUNK],
                            psum_stats[:, lo : lo + CHUNK],
                        )
                        nc.vector.tensor_copy(
                            sb_stats[:, hi : hi + CHUNK],
                            psum_stats[:, hi : hi + CHUNK],
                        )

                    # raw partial stats straight to DRAM; the host reduces.
                    # one partition-strided DMA (rows 0/32/64). Issued on the
                    # ACT HWDGE ring, NOT sync: the SP ring is FIFO, so an
                    # out-DMA waiting on the staging semaphore there would
                    # stall the next iteration's whole input-DMA stream.
                    nc.scalar.dma_start(out_d[:], sb_stats[0:65:32, :])

    nc.compile()
    return nc


class _Exec:
    """Cached PJRT executable for the SPMD kernel (mirrors
    concourse.bass2jax.run_bass_via_pjrt's multi-core branch, but keeps the
    jitted callable so repeat invocations don't recompile)."""

    def __init__(self, nc):
        import jax
        from jax.experimental.shard_map import shard_map
        from jax.sharding import Mesh, PartitionSpec

        from concourse import bass2jax, mybir

        bass2jax.install_neuronx_cc_hook()
        self.nc = nc
        partition_name = (
            nc.partition_id_tensor.name if nc.partition_id_tensor else None
        )

        in_names, out_names, out_avals, zero_outs = [], [], [], []
        for alloc in nc.m.functions[0].allocations:
            if not isinstance(alloc, mybir.MemoryLocationSet):
                continue
            assert alloc.memorylocations
            name = alloc.memorylocations[0].name
            if alloc.kind == "ExternalInput":
                if name != partition_name:
                    in_names.append(name)
            elif alloc.kind == "ExternalOutput":
                shape = tuple(alloc.tensor_shape)
                dtype = mybir.dt.np(alloc.dtype)
                out_names.append(name)
                out_avals.append(jax.core.ShapedArray(shape, dtype))
                zero_outs.append(np.zeros(shape, dtype))

        self.in_names = list(in_names)
        self.out_names = list(out_names)
        self.out_avals = out_avals
        self.zero_outs = zero_outs
        n_params = len(in_names)
        n_outs = len(out_names)

        in_names_full = list(in_names) + list(out_names)
        if partition_name is not None:
            in_names_full.append(partition_name)

        def _body(*args):
            operands = list(args)
            if partition_name is not None:
                operands.append(bass2jax.partition_id_tensor())
            outs = bass2jax._bass_exec_p.bind(
                *operands,
                out_avals=tuple(out_avals),
                in_names=tuple(in_names_full),
                out_names=tuple(out_names),
                lowering_input_output_aliases=(),
                sim_require_finite=True,
                sim_require_nnan=True,
                nc=nc,
            )
            return tuple(outs)

        devices = jax.devices()[:N_CORES]
        assert len(devices) == N_CORES, f"need {N_CORES} devices, got {len(devices)}"
        self.mesh = Mesh(np.asarray(devices), ("core",))
        in_specs = (PartitionSpec("core"),) * (n_params + n_outs)
        out_specs = (PartitionSpec("core"),) * n_outs
        donate = tuple(range(n_params, n_params + n_outs))
        self.sharded = jax.jit(
            shard_map(
                _body,
                mesh=self.mesh,
                in_specs=in_specs,
                out_specs=out_specs,
                check_rep=False,
            ),
            donate_argnums=donate,
            keep_unused=True,
        )

    def concat_zeros(self):
        return [
            np.zeros((N_CORES * z.shape[0], *z.shape[1:]), z.dtype)
            for z in self.zero_outs
        ]

    def run(self, in_map):
        """in_map: name -> full (already concat-along-axis0) array."""
        ins = [in_map[name] for name in self.in_names]
        outs = self.sharded(*ins, *self.concat_zeros())
        return {
            name: np.asarray(outs[i]).reshape(
                N_CORES, *self.out_avals[i].shape
            )
            for i, name in enumerate(self.out_names)
        }


def _get_exec(repeat=1, loop=None):
    key = ("exec", repeat, loop)
    if key not in _CACHE:
        _CACHE[key] = _Exec(_build_nc(repeat, loop=loop))
    return _CACHE[key]


def _epilogue(stats):
    """stats: [N_CORES, 3, 2*D] f32 raw per-core partial sums. Host-side
    unshard: sum partials over cores, then the closed-form loss in f64."""
    st = stats.astype(np.float64).sum(axis=0)
    s1e, s1t = st[0, :D], st[0, D:]
    s2e, s2t = st[1, :D], st[1, D:]
    set_ = st[2, :D]
    n = float(N_FULL)
    var_e = (s2e - s1e * s1e / n) / (n - 1.0)
    var_t = (s2t - s1t * s1t / n) / (n - 1.0)
    std_e = np.maximum(np.sqrt(np.maximum(var_e, 0.0)), EPS)
    std_t = np.maximum(np.sqrt(np.maximum(var_t, 0.0)), EPS)
    cov = set_ - s1e * s1t / n
    c = cov / (std_e * std_t) / (n + EPS)
    c = np.clip(c, -1.0 + EPS, 1.0 - EPS)
    return np.float32(np.sum((1.0 - c) ** 2))


def kernel(e_q, tau):
    e_q = np.ascontiguousarray(np.asarray(e_q), dtype=np.float32)
    tau = np.ascontiguousarray(np.asarray(tau), dtype=np.float32)
    assert e_q.shape == (N_FULL, D) and tau.shape == (N_FULL, D)
    ex = _get_exec()
    # row-sharding across cores: the concatenation of the 8 shards along
    # axis 0 is just the full array, so pass it through unchanged.
    outs = ex.run({"e_q": e_q, "tau": tau})
    return _epilogue(outs["out"])
